# revision 1
# baseline (speedup 1.0000x reference)
"""Causal single-head attention on 8 Trainium2 NeuronCores.

Problem: x [32, 2048, 384] f32, Wq/Wk/Wv [384, 64] f32.
  q/k/v = x @ W;  out = softmax_causal(q k^T / sqrt(64)) @ v   -> [32, 2048, 64]

Strategy: data-parallel over batch (4 batches per core), weights replicated,
bf16 matmul path (fp32 accumulation), no collectives.

Per batch on one core (T=2048, C=384, H=64):
  - host pre-transposes x to xT [C, T] and casts to bf16
  - projection pass 1 with stationary [Wq|Wk] -> psum rows 0:64 = qT,
    rows 64:128 = kT ("hi" copy); pass 2 with [Wv|Wq] -> vT lo + qT hi.
    One extra SBUF->SBUF DMA makes the base-0 kT copy. This gives every
    operand at both partition bases so the causal score matmuls can run as
    row-tiled PAIRS (two K=64 matmuls concurrently in the 128-deep array).
  - scoresT [s, t] pairs write one [128, 2, 512] PSUM strip; one exp
    ACTIVATE per strip (scale=1/8 fused) emits bf16 expT for both chunks
  - AV: outT[65, t] accumulated over s-chunks in PSUM (v-with-ones-column
    stationary, expT moving; row 64 accumulates the softmax denominator)
  - software pipelining: scores of pair p+1 are emitted before AV of pair
    p so the TensorE never head-of-line blocks on the ScalarE exp
  - PE-transpose outT -> [t, 65], reciprocal of col 64, broadcast multiply,
    one DMA per 512-row block, f32 out
"""

import sys

sys.path.insert(0, "/opt/trn_rl_repo")

import numpy as np
import ml_dtypes

import concourse.bass as bass
import concourse.mybir as mybir
import concourse.tile as tile
from concourse import bacc
from concourse.bass_utils import run_bass_kernel_spmd

BF16 = mybir.dt.bfloat16
F32 = mybir.dt.float32
NP_BF16 = ml_dtypes.bfloat16

B, T_FULL, C, H = 32, 2048, 384, 64
N_CORES = 8
B_LOC = B // N_CORES
SCALE = float(H) ** -0.5
Exp = mybir.ActivationFunctionType.Exp


def build_nc(b_loc=B_LOC, t=T_FULL):
    """Build the per-core Bass program (SPMD: same program on all cores)."""
    assert t % 512 == 0
    nc = bacc.Bacc(None, target_bir_lowering=False)
    cc = C // 128          # contraction chunks for projections
    ns = t // 128          # number of 128-wide s-chunks
    nt = t // 512          # number of 512-wide t-chunks
    npair = ns // 2        # s-chunk pairs (even chunk -> base 0, odd -> base 64)

    xT = nc.declare_dram_parameter("xT", [b_loc, C, t], BF16, isOutput=False)
    wq_d = nc.declare_dram_parameter("Wq", [C, H], BF16, isOutput=False)
    wk_d = nc.declare_dram_parameter("Wk", [C, H], BF16, isOutput=False)
    wv_d = nc.declare_dram_parameter("Wv", [C, H], BF16, isOutput=False)
    id16_d = nc.declare_dram_parameter("ident16", [128, 128], BF16, isOutput=False)
    id32_d = nc.declare_dram_parameter("ident32", [128, 128], F32, isOutput=False)
    mask_d = nc.declare_dram_parameter("mask", [128, 128], BF16, isOutput=False)
    outp = nc.declare_dram_parameter("out", [b_loc, t, H], F32, isOutput=True)

    with tile.TileContext(nc) as tc:
        with (
            tc.tile_pool(name="consts", bufs=1) as consts,
            tc.tile_pool(name="xt", bufs=6) as p_xt,
            tc.tile_pool(name="qk", bufs=2) as p_qk,
            tc.tile_pool(name="vv", bufs=2) as p_v,
            tc.tile_pool(name="exp", bufs=3) as p_exp,
            tc.tile_pool(name="oo", bufs=2) as p_o,
            tc.tile_pool(name="ps_big", bufs=2, space="PSUM") as ps_big,
            tc.tile_pool(name="ps_out", bufs=4, space="PSUM") as ps_out,
        ):
            # ---- constants ----
            ident16 = consts.tile([128, 128], BF16)
            nc.sync.dma_start(out=ident16, in_=id16_d[:, :])
            ident32 = consts.tile([128, 128], F32)
            nc.sync.dma_start(out=ident32, in_=id32_d[:, :])
            dmask = consts.tile([128, 128], BF16)
            nc.sync.dma_start(out=dmask, in_=mask_d[:, :])
            # packed projection stationaries: [Wq|Wk] and [Wv|Wq] per c-chunk
            wqk = consts.tile([128, cc, 128], BF16)
            wvq = consts.tile([128, cc, 128], BF16)
            for c in range(cc):
                cs = slice(128 * c, 128 * (c + 1))
                nc.sync.dma_start(out=wqk[:, c, 0:H], in_=wq_d[cs, :])
                nc.sync.dma_start(out=wqk[:, c, H:128], in_=wk_d[cs, :])
                nc.sync.dma_start(out=wvq[:, c, 0:H], in_=wv_d[cs, :])
                nc.sync.dma_start(out=wvq[:, c, H:128], in_=wq_d[cs, :])

            # ---------------- per-batch program ----------------
            def emit_p1(b):
                """Load xT, project q/k/v, build v-augmented tiles.
                Returns (g1, g2, klo, vaug): g1 rows 0:64 = qT lo, rows
                64:128 = kT hi; g2 rows 0:64 = vT, rows 64:128 = qT hi."""
                xts = []
                for c in range(cc):
                    xt_sb = p_xt.tile([128, t], BF16, tag="xt", name=f"xt{b}{c}")
                    nc.sync.dma_start(out=xt_sb, in_=xT[b, 128 * c:128 * (c + 1), :])
                    xts.append(xt_sb)

                g1 = p_qk.tile([128, t], BF16, tag="g1", name=f"g1_{b}")
                g2 = p_qk.tile([128, t], BF16, tag="g2", name=f"g2_{b}")
                for tj in range(nt):
                    tr = slice(512 * tj, 512 * (tj + 1))
                    for w_sb, g_sb in ((wqk, g1), (wvq, g2)):
                        ps_g = ps_big.tile([128, 2, 512], F32, tag="big",
                                           name=f"psg{b}{tj}")
                        for c in range(cc):
                            nc.tensor.matmul(
                                ps_g[:, 0, :], w_sb[:, c, :], xts[c][:, tr],
                                start=(c == 0), stop=(c == cc - 1),
                            )
                        nc.vector.tensor_copy(g_sb[:, tr], ps_g[:, 0, :])
                # base-0 copy of kT (rows 64:128 of g1 -> rows 0:64)
                klo = p_qk.tile([64, t], BF16, tag="klo", name=f"klo{b}")
                nc.sync.dma_start(out=klo, in_=g1[64:128, :])
                # v tiles [s,H] + ones column via PE transpose of vT
                vaug = p_v.tile([128, ns, 65], BF16, tag="vaug", name=f"vaug{b}")
                nc.gpsimd.memset(vaug, 1.0)
                for st in range(ns):
                    ps_tr = ps_big.tile([128, 64], BF16, tag="big",
                                        name=f"pstr{b}{st}")
                    nc.tensor.transpose(
                        ps_tr, g2[0:64, 128 * st:128 * (st + 1)], ident16[0:H, 0:H]
                    )
                    nc.vector.tensor_copy(vaug[:, st, 0:H], ps_tr)
                return g1, g2, klo, vaug

            def emit_scores_pair(b, p, g1, g2, klo):
                """Score matmul pair + exp for s-chunks (2p, 2p+1).
                Returns the bf16 expT tile [128, 2, t]."""
                jd = p // 2
                t0 = 512 * jd
                i0, i1 = 2 * p, 2 * p + 1
                expT = p_exp.tile([128, 2, t], BF16, tag="expT", name=f"exp{b}{p}")
                for sidx, ts0 in enumerate(range(t0, t, 512)):
                    ps_s = ps_big.tile([128, 2, 512], F32, tag="big",
                                       name=f"pss{b}{p}{sidx}")
                    nc.tensor.matmul(
                        ps_s[:, 0, :],
                        klo[:, 128 * i0:128 * (i0 + 1)],
                        g1[0:64, ts0:ts0 + 512],
                        start=True, stop=True,
                    )
                    nc.tensor.matmul(
                        ps_s[:, 1, :],
                        g1[64:128, 128 * i1:128 * (i1 + 1)],
                        g2[64:128, ts0:ts0 + 512],
                        start=True, stop=True,
                    )
                    off = 256 * (p % 2) if sidx == 0 else 0
                    nc.scalar.activation(
                        expT[:, :, ts0 + off:ts0 + 512],
                        ps_s[:, :, off:512],
                        Exp, scale=SCALE,
                    )
                # invalid regions: [t0, 256p) never exp'd (stale), plus the
                # odd chunk's block below its diagonal; diag blocks masked
                if p % 2 == 1:
                    nc.gpsimd.memset(expT[:, :, t0:t0 + 256], 0.0)
                nc.gpsimd.memset(expT[:, 1, 256 * p:256 * p + 128], 0.0)
                d0 = 256 * p
                nc.vector.tensor_mul(
                    expT[:, 0, d0:d0 + 128], expT[:, 0, d0:d0 + 128], dmask
                )
                d1 = 256 * p + 128
                nc.vector.tensor_mul(
                    expT[:, 1, d1:d1 + 128], expT[:, 1, d1:d1 + 128], dmask
                )
                return expT

            def emit_p3(b, j, outT_ps):
                """Transpose outT[65, 512] -> [t,65], normalize, DMA out."""
                outTn = p_o.tile([65, 512], F32, tag="outTn", name=f"otn{b}{j}")
                nc.vector.tensor_copy(outTn, outT_ps)
                ps_o = ps_out.tile([128, 4, 65], F32, tag="outT", name=f"pso{b}{j}")
                for tt in range(4):
                    nc.tensor.transpose(
                        ps_o[:, tt, :],
                        outTn[:, 128 * tt:128 * (tt + 1)],
                        ident32[0:65, 0:65],
                    )
                zrec = p_o.tile([128, 4], F32, tag="zrec", bufs=4, name=f"zr{b}{j}")
                nc.vector.reciprocal(zrec, ps_o[:, :, H:H + 1])
                o_sb = p_o.tile([128, 4, H], F32, tag="o_sb", bufs=4,
                                name=f"os{b}{j}")
                zbc = bass.AP(
                    tensor=zrec.tensor, offset=zrec.offset,
                    ap=[zrec.ap[0], zrec.ap[1], [0, H]],
                )
                nc.vector.tensor_tensor(
                    out=o_sb, in0=ps_o[:, :, 0:H], in1=zbc,
                    op=mybir.AluOpType.mult,
                )
                dst = outp[b, 512 * j:512 * (j + 1), :].rearrange(
                    "(tt tl) h -> tl tt h", tl=128
                )
                nc.sync.dma_start(out=dst, in_=o_sb)

            def emit_av(b, p, i, expT, vaug, outTs):
                """AV accumulation for s-chunk i (parity i%2 of pair p)."""
                jd = i // 4
                for j in range(jd, nt):
                    if i == 0:
                        outTs[j] = ps_out.tile([65, 512], F32, tag="outT",
                                               name=f"outT{b}{j}")
                    nc.tensor.matmul(
                        outTs[j],
                        vaug[:, i, :],
                        expT[:, i % 2, 512 * j:512 * (j + 1)],
                        start=(i == 0), stop=(i == 4 * j + 3),
                    )
                    if i == 4 * j + 3:
                        emit_p3(b, j, outTs[j])

            for b in range(b_loc):
                g1, g2, klo, vaug = emit_p1(b)
                outTs = [None] * nt
                exp_tiles = {}
                exp_tiles[0] = emit_scores_pair(b, 0, g1, g2, klo)
                for p in range(npair):
                    if p + 1 < npair:
                        exp_tiles[p + 1] = emit_scores_pair(b, p + 1, g1, g2, klo)
                    expT = exp_tiles.pop(p)
                    emit_av(b, p, 2 * p, expT, vaug, outTs)
                    emit_av(b, p, 2 * p + 1, expT, vaug, outTs)

    nc.compile()
    return nc


def _shard_inputs(x, Wk, Wq, Wv, b_loc=B_LOC, t=T_FULL):
    ident32 = np.eye(128, dtype=np.float32)
    ident16 = ident32.astype(NP_BF16)
    mask = np.triu(np.ones((128, 128), dtype=np.float32)).astype(NP_BF16)
    wq16 = np.ascontiguousarray(Wq, dtype=np.float32).astype(NP_BF16)
    wk16 = np.ascontiguousarray(Wk, dtype=np.float32).astype(NP_BF16)
    wv16 = np.ascontiguousarray(Wv, dtype=np.float32).astype(NP_BF16)
    n_cores = x.shape[0] // b_loc
    xs = np.asarray(x, dtype=np.float32).reshape(n_cores, b_loc, t, C)
    in_maps = []
    for m in range(n_cores):
        xT = np.ascontiguousarray(xs[m].transpose(0, 2, 1)).astype(NP_BF16)
        in_maps.append({
            "xT": xT, "Wq": wq16, "Wk": wk16, "Wv": wv16,
            "ident16": ident16, "ident32": ident32, "mask": mask,
        })
    return in_maps


def _run(x, Wk, Wq, Wv, trace=False, **spmd_kwargs):
    nc = build_nc()
    in_maps = _shard_inputs(x, Wk, Wq, Wv)
    res = run_bass_kernel_spmd(
        nc, in_maps, core_ids=list(range(N_CORES)), trace=trace, **spmd_kwargs
    )
    out = np.concatenate([res.results[m]["out"] for m in range(N_CORES)], axis=0)
    return np.ascontiguousarray(out, dtype=np.float32), res


def kernel(x, Wk, Wq, Wv):
    out, _ = _run(x, Wk, Wq, Wv)
    return out



# revision 2
# speedup vs baseline: 1.2123x; 1.2123x over previous
"""Causal single-head attention on 8 Trainium2 NeuronCores.

Problem: x [32, 2048, 384] f32, Wq/Wk/Wv [384, 64] f32.
  q/k/v = x @ W;  out = softmax_causal(q k^T / sqrt(64)) @ v   -> [32, 2048, 64]

Strategy: data-parallel over batch (4 batches per core), weights replicated,
bf16 matmul path (fp32 accumulation), no collectives.

Per batch on one core (T=2048, C=384, H=64):
  - host pre-transposes x to xT [C, T] and casts to bf16
  - projection pass 1 with stationary [Wq|Wk] -> psum rows 0:64 = qT,
    rows 64:128 = kT ("hi" copy); pass 2 with [Wv|Wq] -> vT lo + qT hi.
    One extra SBUF->SBUF DMA makes the base-0 kT copy. This gives every
    operand at both partition bases so the causal score matmuls can run as
    row-tiled PAIRS (two K=64 matmuls concurrently in the 128-deep array).
  - score strips are produced in DIAGONAL order (strip (p, s) covers
    t-chunk p//2 + s; diagonal d = all strips covering t-chunk d), exp'd
    per strip (scale=1/8 fused), with matmul N trimmed to the causal
    region at 128 granularity (no memsets needed; garbage cols are never
    read by AV)
  - AV is t-chunk-major: for t-chunk j accumulate s-chunks 0..4j+3 into
    one PSUM bank [65, 512] (v-with-ones-column stationary, row 64 =
    softmax denominator); per-s-chunk start offset at 128 granularity
  - emission interleaves: diag d strips, AV j=d-1, and next batch's
    projections are woven so TensorE always has work while ScalarE
    (the exp pacer) drains strips; PSUM: 2x strip (2 banks each),
    2x proj (1 bank), 2x AV-accum/transpose (1 bank) = 8 banks
  - PE-transpose outT -> [t, 65], reciprocal of col 64, broadcast multiply,
    one DMA per 512-row block, f32 out
  - optional: a subset of strips (GP_STRIPS) does exp on GpSimd via the
    Schraudolph bit trick (i16 = round(A*score + B) viewed as bf16),
    offloading the ScalarE bottleneck
"""

import sys

sys.path.insert(0, "/opt/trn_rl_repo")

import numpy as np
import ml_dtypes

import concourse.bass as bass
import concourse.mybir as mybir
import concourse.tile as tile
from concourse import bacc
from concourse.bass_utils import run_bass_kernel_spmd

BF16 = mybir.dt.bfloat16
F32 = mybir.dt.float32
I16 = mybir.dt.int16
NP_BF16 = ml_dtypes.bfloat16

B, T_FULL, C, H = 32, 2048, 384, 64
N_CORES = 8
B_LOC = B // N_CORES
SCALE = float(H) ** -0.5
Exp = mybir.ActivationFunctionType.Exp

# Schraudolph exp-to-bf16 constants: bf16_bits(exp(s*SCALE)) ~
# round(A_S * s + B_S) as int16.  C_S tunes the mean relative error.
C_S = 8.5
A_S = 128.0 / np.log(2.0) * SCALE
B_S = 127.0 * 128.0 - C_S

# strips (p, sidx) whose exp runs on GpSimd (Schraudolph) instead of ScalarE
GP_STRIPS = set()


def build_nc(b_loc=B_LOC, t=T_FULL):
    """Build the per-core Bass program (SPMD: same program on all cores)."""
    assert t % 512 == 0
    nc = bacc.Bacc(None, target_bir_lowering=False)
    cc = C // 128          # contraction chunks for projections
    ns = t // 128          # number of 128-wide s-chunks
    nt = t // 512          # number of 512-wide t-chunks
    npair = ns // 2        # s-chunk pairs (even chunk -> base 0, odd -> base 64)

    xT = nc.declare_dram_parameter("xT", [b_loc, C, t], BF16, isOutput=False)
    wq_d = nc.declare_dram_parameter("Wq", [C, H], BF16, isOutput=False)
    wk_d = nc.declare_dram_parameter("Wk", [C, H], BF16, isOutput=False)
    wv_d = nc.declare_dram_parameter("Wv", [C, H], BF16, isOutput=False)
    id16_d = nc.declare_dram_parameter("ident16", [128, 128], BF16, isOutput=False)
    id32_d = nc.declare_dram_parameter("ident32", [128, 128], F32, isOutput=False)
    mask_d = nc.declare_dram_parameter("mask", [128, 128], BF16, isOutput=False)
    outp = nc.declare_dram_parameter("out", [b_loc, t, H], F32, isOutput=True)

    with tile.TileContext(nc) as tc:
        with (
            tc.tile_pool(name="consts", bufs=1) as consts,
            tc.tile_pool(name="xt", bufs=6) as p_xt,
            tc.tile_pool(name="qk", bufs=2) as p_qk,
            tc.tile_pool(name="vv", bufs=2) as p_v,
            tc.tile_pool(name="exp", bufs=3) as p_exp,
            tc.tile_pool(name="oo", bufs=2) as p_o,
            tc.tile_pool(name="ps_strip", bufs=2, space="PSUM") as ps_strip,
            tc.tile_pool(name="ps_proj", bufs=2, space="PSUM") as ps_proj,
            tc.tile_pool(name="ps_av", bufs=2, space="PSUM") as ps_av,
        ):
            # ---- constants ----
            ident16 = consts.tile([128, 128], BF16)
            nc.sync.dma_start(out=ident16, in_=id16_d[:, :])
            ident32 = consts.tile([128, 128], F32)
            nc.sync.dma_start(out=ident32, in_=id32_d[:, :])
            dmask = consts.tile([128, 128], BF16)
            nc.sync.dma_start(out=dmask, in_=mask_d[:, :])
            # packed projection stationaries: [Wq|Wk] and [Wv|Wq] per c-chunk
            wqk = consts.tile([128, cc, 128], BF16)
            wvq = consts.tile([128, cc, 128], BF16)
            for c in range(cc):
                cs = slice(128 * c, 128 * (c + 1))
                nc.sync.dma_start(out=wqk[:, c, 0:H], in_=wq_d[cs, :])
                nc.sync.dma_start(out=wqk[:, c, H:128], in_=wk_d[cs, :])
                nc.sync.dma_start(out=wvq[:, c, 0:H], in_=wv_d[cs, :])
                nc.sync.dma_start(out=wvq[:, c, H:128], in_=wq_d[cs, :])

            # per-batch live state
            state = {}

            def emit_xt_dma(b):
                xts = []
                for c in range(cc):
                    xt_sb = p_xt.tile([128, t], BF16, tag="xt", name=f"xt{b}{c}")
                    nc.sync.dma_start(out=xt_sb, in_=xT[b, 128 * c:128 * (c + 1), :])
                    xts.append(xt_sb)
                return xts

            def emit_proj(b, xts):
                """Project q/k/v, build v-augmented tiles.
                g1 rows 0:64 = qT lo, rows 64:128 = kT hi;
                g2 rows 0:64 = vT, rows 64:128 = qT hi."""
                g1 = p_qk.tile([128, t], BF16, tag="g1", name=f"g1_{b}")
                g2 = p_qk.tile([128, t], BF16, tag="g2", name=f"g2_{b}")
                for tj in range(nt):
                    tr = slice(512 * tj, 512 * (tj + 1))
                    for w_sb, g_sb in ((wqk, g1), (wvq, g2)):
                        ps_g = ps_proj.tile([128, 512], F32, tag="proj",
                                            name=f"psg{b}{tj}")
                        for c in range(cc):
                            nc.tensor.matmul(
                                ps_g, w_sb[:, c, :], xts[c][:, tr],
                                start=(c == 0), stop=(c == cc - 1),
                            )
                        nc.vector.tensor_copy(g_sb[:, tr], ps_g)
                # base-0 copy of kT (rows 64:128 of g1 -> rows 0:64)
                klo = p_qk.tile([64, t], BF16, tag="klo", name=f"klo{b}")
                nc.sync.dma_start(out=klo, in_=g1[64:128, :])
                # v tiles [s,H] + ones column via PE transpose of vT
                vaug = p_v.tile([128, ns, 65], BF16, tag="vaug", name=f"vaug{b}")
                nc.gpsimd.memset(vaug[:, :, H:H + 1], 1.0)
                for st in range(ns):
                    ps_tr = ps_proj.tile([128, 64], BF16, tag="proj",
                                         name=f"pstr{b}{st}")
                    nc.tensor.transpose(
                        ps_tr, g2[0:64, 128 * st:128 * (st + 1)], ident16[0:H, 0:H]
                    )
                    nc.vector.tensor_copy(vaug[:, st, 0:H], ps_tr)
                state[b] = dict(g1=g1, g2=g2, klo=klo, vaug=vaug,
                                expT={}, outT={})

            def emit_strip(b, p, sidx):
                """Score matmul pair + exp for strip sidx of pair p.
                Strip covers t-chunk p//2 + sidx."""
                st = state[b]
                g1, g2, klo = st["g1"], st["g2"], st["klo"]
                jd = p // 2
                t0 = 512 * jd
                i0, i1 = 2 * p, 2 * p + 1
                w = t - t0            # expT tile width for this pair
                if sidx == 0:
                    expT = p_exp.tile([128, 2, w], BF16, tag=f"exp{jd}",
                                      name=f"exp{b}_{p}")
                    st["expT"][p] = expT
                else:
                    expT = st["expT"][p]
                ts0 = t0 + 512 * sidx
                ps_s = ps_strip.tile([128, 2, 512], F32, tag="strip",
                                     name=f"pss{b}{p}{sidx}")
                if sidx == 0:
                    # causal trim: chunk i valid from col 128*i (abs t)
                    off0 = 128 * i0 - ts0        # 0 (even p) / 256 (odd p)
                    off1 = 128 * i1 - ts0        # 128 (even p) / 384 (odd p)
                else:
                    off0 = off1 = 0
                nc.tensor.matmul(
                    ps_s[:, 0, off0:512],
                    klo[:, 128 * i0:128 * (i0 + 1)],
                    g1[0:64, ts0 + off0:ts0 + 512],
                    start=True, stop=True,
                )
                nc.tensor.matmul(
                    ps_s[:, 1, off1:512],
                    g1[64:128, 128 * i1:128 * (i1 + 1)],
                    g2[64:128, ts0 + off1:ts0 + 512],
                    start=True, stop=True,
                )
                # exp over the union of valid cols (parity-1 cols in
                # [off0, off1) are garbage but never read by AV)
                eoff = off0
                dst = expT[:, :, ts0 - t0 + eoff:ts0 - t0 + 512]
                src = ps_s[:, :, eoff:512]
                if (p, sidx) in GP_STRIPS:
                    nc.gpsimd.tensor_scalar(
                        dst.bitcast(I16), src, A_S, B_S,
                        op0=mybir.AluOpType.mult, op1=mybir.AluOpType.add,
                    )
                else:
                    nc.scalar.activation(dst, src, Exp, scale=SCALE)
                if sidx == 0:
                    # mask the two diagonal blocks
                    d0 = 128 * i0 - t0
                    nc.vector.tensor_mul(
                        expT[:, 0, d0:d0 + 128], expT[:, 0, d0:d0 + 128], dmask
                    )
                    d1 = 128 * i1 - t0
                    nc.vector.tensor_mul(
                        expT[:, 1, d1:d1 + 128], expT[:, 1, d1:d1 + 128], dmask
                    )

            def emit_p3(b, j, outT_ps):
                """Transpose outT[65, 512] -> [t,65], normalize, DMA out."""
                outTn = p_o.tile([65, 512], F32, tag="outTn", name=f"otn{b}{j}")
                nc.vector.tensor_copy(outTn, outT_ps)
                ps_o = ps_av.tile([128, 4, 65], F32, tag="av", name=f"pso{b}{j}")
                for tt in range(4):
                    nc.tensor.transpose(
                        ps_o[:, tt, :],
                        outTn[:, 128 * tt:128 * (tt + 1)],
                        ident32[0:65, 0:65],
                    )
                zrec = p_o.tile([128, 4], F32, tag="zrec", bufs=4, name=f"zr{b}{j}")
                nc.vector.reciprocal(zrec, ps_o[:, :, H:H + 1])
                o_sb = p_o.tile([128, 4, H], F32, tag="o_sb", bufs=4,
                                name=f"os{b}{j}")
                zbc = bass.AP(
                    tensor=zrec.tensor, offset=zrec.offset,
                    ap=[zrec.ap[0], zrec.ap[1], [0, H]],
                )
                nc.vector.tensor_tensor(
                    out=o_sb, in0=ps_o[:, :, 0:H], in1=zbc,
                    op=mybir.AluOpType.mult,
                )
                dst = outp[b, 512 * j:512 * (j + 1), :].rearrange(
                    "(tt tl) h -> tl tt h", tl=128
                )
                nc.sync.dma_start(out=dst, in_=o_sb)

            def emit_av(b, j):
                """AV accumulation for t-chunk j over s-chunks 0..4j+3."""
                st = state[b]
                vaug = st["vaug"]
                outT = ps_av.tile([65, 512], F32, tag="av", name=f"outT{b}{j}")
                n_i = 4 * j + 4
                for i in range(n_i):
                    p = i // 2
                    t0 = 512 * (p // 2)
                    expT = st["expT"][p]
                    # causal trim: chunk i contributes from col 128*i
                    off = max(0, 128 * i - 512 * j)
                    c0 = 512 * j + off - t0
                    nc.tensor.matmul(
                        outT[:, off:512],
                        vaug[:, i, :],
                        expT[:, i % 2, c0:c0 + 512 - off],
                        start=(i == 0), stop=(i == n_i - 1),
                    )
                emit_p3(b, j, outT)

            # ---------------- schedule ----------------
            # diagonal d of batch b = strips (p, d - p//2) for p//2 <= d
            def diag_strips(d):
                return [(p, d - p // 2) for p in range(2 * d + 2)]

            # software-pipelined order: weave next batch's projections and
            # first diagonal into this batch's tail so ScalarE never idles
            for b in range(b_loc):
                if b == 0:
                    xts = emit_xt_dma(0)
                    emit_proj(0, xts)
                for d in range(nt):
                    for (p, sidx) in diag_strips(d):
                        emit_strip(b, p, sidx)
                    if d == 1:
                        emit_av(b, 0)
                    elif d == 2:
                        emit_av(b, 1)
                    elif d == 3:
                        if b + 1 < b_loc:
                            xts = emit_xt_dma(b + 1)
                        emit_av(b, 2)
                if b + 1 < b_loc:
                    emit_proj(b + 1, xts)
                    for (p, sidx) in diag_strips(0):
                        emit_strip(b + 1, p, sidx)
                emit_av(b, 3)
                del state[b]

    nc.compile()
    return nc


def _shard_inputs(x, Wk, Wq, Wv, b_loc=B_LOC, t=T_FULL):
    ident32 = np.eye(128, dtype=np.float32)
    ident16 = ident32.astype(NP_BF16)
    mask = np.triu(np.ones((128, 128), dtype=np.float32)).astype(NP_BF16)
    wq16 = np.ascontiguousarray(Wq, dtype=np.float32).astype(NP_BF16)
    wk16 = np.ascontiguousarray(Wk, dtype=np.float32).astype(NP_BF16)
    wv16 = np.ascontiguousarray(Wv, dtype=np.float32).astype(NP_BF16)
    n_cores = x.shape[0] // b_loc
    xs = np.asarray(x, dtype=np.float32).reshape(n_cores, b_loc, t, C)
    in_maps = []
    for m in range(n_cores):
        xT = np.ascontiguousarray(xs[m].transpose(0, 2, 1)).astype(NP_BF16)
        in_maps.append({
            "xT": xT, "Wq": wq16, "Wk": wk16, "Wv": wv16,
            "ident16": ident16, "ident32": ident32, "mask": mask,
        })
    return in_maps


def _run(x, Wk, Wq, Wv, trace=False, **spmd_kwargs):
    nc = build_nc()
    in_maps = _shard_inputs(x, Wk, Wq, Wv)
    res = run_bass_kernel_spmd(
        nc, in_maps, core_ids=list(range(N_CORES)), trace=trace, **spmd_kwargs
    )
    out = np.concatenate([res.results[m]["out"] for m in range(N_CORES)], axis=0)
    return np.ascontiguousarray(out, dtype=np.float32), res


def kernel(x, Wk, Wq, Wv):
    out, _ = _run(x, Wk, Wq, Wv)
    return out
